# revision 1
# baseline (speedup 1.0000x reference)
"""Trainium2 Bass kernel for ConnectomeNetwork (gnn_message_passing).

Computation (reference):
    out = x @ W_retina^T                      # [B, N], nonzero only at visual cols
    for _ in range(n_layers): out = out @ W_shared^T
    y = out @ W_rational^T                    # [B, 2], reads only rational cols

Strategy (8 NeuronCores, tensor-parallel over output columns):
  * Host-side exact restructuring:
      - vis = nonzero rows of W_retina  -> layer 1 contracts over |vis| dims only.
      - last connectome matmul + rational readout fold into
        M = W_rational[:, rat] @ W_shared[rat, :]  (only |rat| rows of W touched).
  * Each core computes a 1/8 column-shard of every layer; activations are
    kept transposed in SBUF ([n, B] tiles) so they serve as the stationary
    matmul operand; AllGather (bf16) redistributes activations between full
    layers.  The final layer produces per-core [B, 2] partials summed on host.
  * Weights are pre-transposed/sharded/slab-swizzled/cast to bf16 on host;
    fp32 PSUM accumulation.  A tail portion of the shared weights is pinned
    in SBUF and reused by both middle layers; the rest streams through a
    deep DMA ring that prefetches across the AllGather latency.
"""

import contextlib
import ctypes
import os

import numpy as np
import ml_dtypes

NCORES = 8
PART = 128
SLAB_KC = 4              # 128-row k-chunks per weight DMA slab
PIN_SLABS = 5            # W2 tail slabs pinned in SBUF across both mid layers
STREAM_BUFS = 8          # weight stream ring depth (prefetch runway)
RET_MSLAB = 1024         # retina output columns per resident slab

bf16_np = ml_dtypes.bfloat16

_compiled_cache = {}


# --------------------------------------------------------------------------
# optional NTFF profiling hook (active only when BASS_KERNEL_PROFILE_DIR set)
# --------------------------------------------------------------------------
def _profile_ctx():
    out_dir = os.environ.get("BASS_KERNEL_PROFILE_DIR")
    if not out_dir:
        return contextlib.nullcontext()
    try:
        lib = ctypes.CDLL("/opt/axon/libaxon_pjrt.so")
        if not hasattr(lib, "axon_start_nrt_profile"):
            return contextlib.nullcontext()
        lib.axon_start_nrt_profile.argtypes = [
            ctypes.POINTER(ctypes.c_int64),
            ctypes.c_size_t,
        ]
        lib.axon_start_nrt_profile.restype = ctypes.c_int64
        lib.axon_stop_nrt_profile.argtypes = [ctypes.c_char_p]
        lib.axon_stop_nrt_profile.restype = ctypes.c_int64
    except OSError:
        return contextlib.nullcontext()

    @contextlib.contextmanager
    def _hook():
        import jax

        jax.devices()
        ids_env = os.environ.get("BASS_KERNEL_PROFILE_CORES", "")
        if ids_env:
            ids_list = [int(t) for t in ids_env.split(",") if t != ""]
            ids = (ctypes.c_int64 * len(ids_list))(*ids_list)
            rc = lib.axon_start_nrt_profile(ids, len(ids_list))
        else:
            rc = lib.axon_start_nrt_profile(None, 0)
        if rc != 0:
            raise RuntimeError(f"axon_start_nrt_profile rc={rc}")
        try:
            yield
        finally:
            os.makedirs(out_dir, exist_ok=True)
            n = lib.axon_stop_nrt_profile(str(out_dir).encode())
            print(f"profile: {n} file(s) written to {out_dir}")

    return _hook()


def _ensure_axon_platform():
    """Make sure jax exposes the trn2 NeuronCores (if something switched jax
    to cpu in this process, switch back to the default platform)."""
    import jax

    try:
        devs = jax.devices()
    except Exception:
        devs = []
    if len(devs) >= NCORES and all("cpu" not in str(d).lower() for d in devs[:NCORES]):
        return
    import jax.extend.backend as jeb

    jeb.clear_backends()
    jax.config.update("jax_platforms", None)
    devs = jax.devices()
    if len(devs) < NCORES:
        raise RuntimeError(f"need {NCORES} neuron cores, got {devs}")


# --------------------------------------------------------------------------
# device program
# --------------------------------------------------------------------------
def _build_program(B, R, N, NVIS, n_layers):
    """Build the SPMD Bass program (identical on all cores; per-core data)."""
    import concourse.bacc as bacc
    import concourse.tile as tile
    import concourse.mybir as mybir

    bf16 = mybir.dt.bfloat16
    f32 = mybir.dt.float32

    MSH = N // NCORES          # column shard per core
    KC_R = R // PART           # k-chunks of the retina contraction
    KC_VIS = NVIS // PART      # k-chunks of layer-1 contraction
    KC_N = N // PART           # k-chunks of a full layer contraction
    MT = MSH // PART           # 128-row tiles in the local column shard
    NMC = MSH // 512           # 512-wide psum chunks per shard
    assert MSH % 512 == 0 and N % PART == 0 and R % PART == 0
    ROWS_SLAB = SLAB_KC * PART

    n_full = KC_N // SLAB_KC                     # slabs in a full layer
    has_mid = n_layers >= 3
    n_pin = min(PIN_SLABS, n_full) if has_mid else 0
    n_stream2 = n_full - n_pin                   # streamed slabs of w2
    n_s1 = (KC_VIS + SLAB_KC - 1) // SLAB_KC     # slabs of w1

    n_ags = max(n_layers - 2, 0)

    nc = bacc.Bacc("TRN2", target_bir_lowering=False, debug=False,
                   num_devices=NCORES)

    xT_d = nc.dram_tensor("xT", [R, B], bf16, kind="ExternalInput")
    wret_d = nc.dram_tensor("wret", [R, NVIS], bf16, kind="ExternalInput")
    # slab-swizzled: [slab, partition, SLAB_KC * MSH] (contiguous per partition)
    w1_d = nc.dram_tensor("w1", [n_s1, PART, SLAB_KC * MSH], bf16,
                          kind="ExternalInput")
    if has_mid:
        if n_stream2:
            w2s_d = nc.dram_tensor("w2s", [n_stream2, PART, SLAB_KC * MSH],
                                   bf16, kind="ExternalInput")
        if n_pin:
            w2p_d = nc.dram_tensor("w2p", [n_pin, PART, SLAB_KC * MSH],
                                   bf16, kind="ExternalInput")
    m4_d = nc.dram_tensor("m4", [MSH, 2], bf16, kind="ExternalInput")
    y_d = nc.dram_tensor("y_part", [B, 2], f32, kind="ExternalOutput")

    ag_outs = [
        nc.dram_tensor(f"ag_out{i}", [NCORES, PART, MT * B], bf16,
                       addr_space="Shared")
        for i in range(n_ags)
    ]
    with tile.TileContext(nc) as tc:
        with (
            tc.tile_pool(name="const", bufs=1) as const,
            tc.tile_pool(name="acts", bufs=1) as acts,
            tc.tile_pool(name="wstream", bufs=STREAM_BUFS) as wstream,
            tc.tile_pool(name="wpinp", bufs=1) as wpinp,
            tc.tile_pool(name="ypool", bufs=2) as ypool,
            tc.tile_pool(name="stage", bufs=2) as stage,
            tc.tile_pool(name="psum", bufs=1, space="PSUM") as psum,
            tc.tile_pool(name="dram", bufs=1, space="DRAM") as dram,
        ):
            # ---- constants ------------------------------------------------
            xT_sb = const.tile([PART, KC_R, B], bf16, name="xT_sb")
            nc.sync.dma_start(
                xT_sb[:], xT_d.rearrange("(kc p) b -> p kc b", p=PART))
            m4_sb = const.tile([PART, MT, 2], bf16, name="m4_sb")
            nc.sync.dma_start(
                m4_sb[:], m4_d.rearrange("(t p) o -> p t o", p=PART))

            # ---- retina: aTvis[n, b] = (x @ Wret^T)^T, n in vis ----------
            aTvis = acts.tile([PART, KC_VIS, B], bf16, name="aTvis")
            n_rslab = (NVIS + RET_MSLAB - 1) // RET_MSLAB
            for rs in range(n_rslab):
                m0 = rs * RET_MSLAB
                msz = min(RET_MSLAB, NVIS - m0)
                wret_sb = wstream.tile([PART, KC_R, RET_MSLAB], bf16,
                                       name="wret_sb", tag="wret_slab", bufs=1)
                nc.sync.dma_start(
                    wret_sb[:, :, :msz],
                    wret_d[:, m0:m0 + msz].rearrange(
                        "(kc p) m -> p kc m", p=PART))
                for mt in range(msz // PART):
                    ps_r = psum.tile([PART, B], f32, name="ps_r",
                                     tag="ps_ret", bufs=2)
                    for kc in range(KC_R):
                        nc.tensor.matmul(
                            ps_r[:],
                            wret_sb[:, kc, mt * PART:(mt + 1) * PART],
                            xT_sb[:, kc, :],
                            start=(kc == 0), stop=(kc == KC_R - 1))
                    nc.vector.tensor_copy(
                        aTvis[:, (m0 // PART) + mt, :], ps_r[:])

            # ---- connectome layers ---------------------------------------
            def w_layer(aT_in, n_kc, stream_d, n_stream, pins, out_aT):
                """out_aT[:, t, b] = cast_bf16((act @ Wshard^T)^T), local shard.

                stream_d: slab-swizzled dram weights, n_stream slabs, streamed
                pins: list of resident SBUF slab tiles used for the k-tail
                """
                psums = [
                    psum.tile([B, 512], f32, name=f"ps_mm{mc}",
                              tag=f"ps_mm{mc}")
                    for mc in range(NMC)
                ]

                n_mm_total = n_kc * NMC
                mm_count = 0

                def mm_slab(wt, sl, nk):
                    nonlocal mm_count
                    for kc in range(nk):
                        kg = sl * SLAB_KC + kc
                        for mc in range(NMC):
                            nc.tensor.matmul(
                                psums[mc][:],
                                aT_in[:, kg, :],
                                wt[:, kc, mc * 512:(mc + 1) * 512],
                                start=(mm_count < NMC),
                                stop=(mm_count >= n_mm_total - NMC))
                            mm_count += 1

                for sl in range(n_stream):
                    nk = min(SLAB_KC, n_kc - sl * SLAB_KC)
                    wt = wstream.tile([PART, SLAB_KC, MSH], bf16, name="wt",
                                      tag="w_slab")
                    nc.sync.dma_start(
                        wt.rearrange("p kc m -> p (kc m)")[:, :nk * MSH],
                        stream_d[sl][:, :nk * MSH])
                    mm_slab(wt, sl, nk)
                for i, wp in enumerate(pins):
                    mm_slab(wp, n_stream + i, SLAB_KC)

                y_sb = ypool.tile([B, MSH], bf16, name="y_sb", tag="y_sb")
                for mc in range(NMC):
                    nc.vector.tensor_copy(
                        y_sb[:, mc * 512:(mc + 1) * 512], psums[mc][:])
                # transpose [B, MSH] -> [MSH, B] via 32x32 DVE blocks
                for t in range(MT):
                    for i in range(4):
                        nc.vector.transpose(
                            out_aT[32 * i:32 * (i + 1), t, :],
                            y_sb[:, t * PART + 32 * i:t * PART + 32 * (i + 1)])

            pins = []

            def load_pins():
                # pinned W2 tail slabs (loaded once, shared by all mid
                # layers); emitted after L1 so they don't compete with L1's
                # critical slabs and instead fill the AG1 DMA window
                for s in range(n_pin):
                    wp = wpinp.tile([PART, SLAB_KC, MSH], bf16,
                                    name=f"wpin{s}", tag=f"wpin{s}")
                    nc.sync.dma_start(
                        wp.rearrange("p kc m -> p (kc m)"), w2p_d[s])
                    pins.append(wp)

            aT = aTvis
            n_kc = KC_VIS
            first = True
            for li in range(n_layers - 1):
                last = (li == n_layers - 2)
                if first:
                    args = (w1_d, n_s1, [])
                else:
                    args = (w2s_d if n_stream2 else None, n_stream2, pins)
                if last:
                    aT_loc = acts.tile([PART, MT, B], bf16, name="aT_loc",
                                       tag="aT_loc")
                    w_layer(aT, n_kc, *args, aT_loc)
                    aT = aT_loc
                    n_kc = MT
                else:
                    aT_stage = stage.tile([PART, MT, B], bf16, name="aT_stage",
                                          tag="aT_stage")
                    w_layer(aT, n_kc, *args, aT_stage)
                    if first:
                        load_pins()
                    # collective plumbing on scalar (HWDGE, idle engine):
                    # keeps the Sync weight-stream queue free of cc waits
                    cc_in = dram.tile([PART, MT * B], bf16, name="cc_in",
                                      tag="cc_in", bufs=2)
                    nc.scalar.dma_start(
                        cc_in[:], aT_stage.rearrange("p t b -> p (t b)"))
                    nc.gpsimd.collective_compute(
                        "AllGather",
                        mybir.AluOpType.bypass,
                        replica_groups=[list(range(NCORES))],
                        ins=[cc_in.opt()],
                        outs=[ag_outs[li][:]],
                    )
                    aT_full = acts.tile([PART, KC_N, B], bf16,
                                        name="aT_full", tag="aT_full")
                    nc.scalar.dma_start(
                        aT_full.rearrange("p (r t) b -> p r t b", t=MT),
                        ag_outs[li].rearrange("r p (t b) -> p r t b", b=B))
                    aT = aT_full
                    n_kc = KC_N
                first = False

            # ---- folded last layer + rational readout --------------------
            ps4 = psum.tile([B, 2], f32, name="ps4", tag="ps4")
            for t in range(n_kc):
                nc.tensor.matmul(
                    ps4[:], aT[:, t, :], m4_sb[:, t, :],
                    start=(t == 0), stop=(t == n_kc - 1))
            y_sb4 = ypool.tile([B, 2], f32, name="y_sb4", tag="y4")
            nc.vector.tensor_copy(y_sb4[:], ps4[:])
            nc.sync.dma_start(y_d[:], y_sb4[:])

    nc.compile()
    return nc


def _slab_swizzle(w, slab_kc):
    """[rows, m] -> [n_slabs, 128, slab_kc * m] with per-partition-contiguous
    slab layout: out[s, p, j*m : (j+1)*m] = w[(s*slab_kc + j)*128 + p, :].
    Rows are zero-padded up to a whole slab."""
    rows, m = w.shape
    n_slabs = (rows + slab_kc * PART - 1) // (slab_kc * PART)
    pad_rows = n_slabs * slab_kc * PART - rows
    if pad_rows:
        w = np.concatenate([w, np.zeros((pad_rows, m), w.dtype)], axis=0)
    out = (w.reshape(n_slabs, slab_kc, PART, m)
           .transpose(0, 2, 1, 3)
           .reshape(n_slabs, PART, slab_kc * m))
    return np.ascontiguousarray(out)


# --------------------------------------------------------------------------
# host entry point
# --------------------------------------------------------------------------
def kernel(x, W_retina, W_shared, W_rational, n_layers):
    x = np.asarray(x, np.float32)
    W_retina = np.asarray(W_retina, np.float32)
    W_shared = np.asarray(W_shared, np.float32)
    W_rational = np.asarray(W_rational, np.float32)
    L = int(n_layers)

    B, R = x.shape
    N = W_shared.shape[0]
    O = W_rational.shape[0]

    vis = np.flatnonzero(np.any(W_retina != 0, axis=1))
    rat = np.flatnonzero(np.any(W_rational != 0, axis=0))

    if L < 2 or len(vis) == 0 or len(rat) == 0 or N % (NCORES * 512) != 0 \
            or R % PART != 0 or O != 2:
        # degenerate configs: exact numpy fallback
        out = x @ W_retina.T
        for _ in range(L):
            out = out @ W_shared.T
        return (out @ W_rational.T).astype(np.float32)

    # pad vis with zero-rows of W_retina (exact: they contribute 0)
    pad = (-len(vis)) % PART
    if pad:
        comp = np.setdiff1d(np.arange(N), vis, assume_unique=False)
        vis = np.concatenate([vis, comp[:pad]])
    NVIS = len(vis)
    MSH = N // NCORES
    KC_N = N // PART
    n_full = KC_N // SLAB_KC
    has_mid = L >= 3
    n_pin = min(PIN_SLABS, n_full) if has_mid else 0
    n_stream2 = n_full - n_pin
    split_row = n_stream2 * SLAB_KC * PART

    # ---- host-side weight prep (bf16, transposed, sharded, swizzled) -----
    xT = np.ascontiguousarray(x.T).astype(bf16_np)                    # [R, B]
    wret = np.ascontiguousarray(W_retina[vis].T).astype(bf16_np)      # [R, NVIS]
    # folded last layer: M = W_rational[:, rat] @ W_shared[rat, :]  -> [O, N]
    M = (W_rational[:, rat].astype(np.float64)
         @ W_shared[rat, :].astype(np.float64)).astype(np.float32)

    w1_c, w2s_c, w2p_c, m4_c = [], [], [], []
    for c in range(NCORES):
        sl = slice(c * MSH, (c + 1) * MSH)
        Ws = W_shared[sl, :]                                          # [MSH, N]
        WsT = np.ascontiguousarray(Ws.T).astype(bf16_np)              # [N, MSH]
        w1_c.append(_slab_swizzle(
            np.ascontiguousarray(Ws[:, vis].T).astype(bf16_np), SLAB_KC))
        if has_mid:
            w2s_c.append(_slab_swizzle(WsT[:split_row], SLAB_KC))
            if n_pin:
                w2p_c.append(_slab_swizzle(WsT[split_row:], SLAB_KC))
        m4_c.append(np.ascontiguousarray(M[:, sl].T).astype(bf16_np))

    _ensure_axon_platform()
    from concourse.bass_utils import run_bass_kernel_spmd

    key = (B, R, N, NVIS, L)
    if key not in _compiled_cache:
        _compiled_cache[key] = _build_program(B, R, N, NVIS, L)
    nc = _compiled_cache[key]

    in_maps = []
    for c in range(NCORES):
        m = {"xT": xT, "wret": wret, "w1": w1_c[c], "m4": m4_c[c]}
        if has_mid:
            if n_stream2:
                m["w2s"] = w2s_c[c]
            if n_pin:
                m["w2p"] = w2p_c[c]
        in_maps.append(m)

    with _profile_ctx():
        res = run_bass_kernel_spmd(nc, in_maps, core_ids=list(range(NCORES)))

    y = np.zeros((B, O), np.float64)
    for c in range(NCORES):
        y += res.results[c]["y_part"].astype(np.float64)
    return y.astype(np.float32)



# revision 3
# speedup vs baseline: 1.2828x; 1.2828x over previous
"""Trainium2 Bass kernel for ConnectomeNetwork (gnn_message_passing).

Computation (reference):
    out = x @ W_retina^T                      # [B, N], nonzero only at visual cols
    for _ in range(n_layers): out = out @ W_shared^T
    y = out @ W_rational^T                    # [B, 2]

Strategy (8 NeuronCores, tensor-parallel over output columns):
  * Host folds:  F = W_shared[:, vis] @ W_retina[vis, :]   (retina + layer 1)
                 M = W_rational[:, rat] @ W_shared[rat, :] (layer L + rational)
    so the device runs L-1 dense layers: one bf16 layer contracting over
    R=1024 (F), then L-2 "mid" layers over the full N contraction, then a
    tiny folded readout.
  * Mid layers run in fp8 (e4m3) with DoubleRow matmuls: weights are scaled
    by 8, activations are rescaled per layer with static power-of-2 scales
    derived from a weight-only random probe.  Quantization noise injected
    before the last layer is strongly attenuated because W_shared's spectrum
    is dominated by its all-positive mean direction.
  * Each core owns a 1536-column shard of every layer; its fp8 weight shard
    (18.9 MB) streams into SBUF once and stays pinned for both mid layers.
    Activations are AllGathered (fp8 payload) between layers in 2 chunks,
    overlapped with compute where the dataflow allows.
  * The folded first layer is computed transposed (F stationary, xT moving)
    so activations come out of PSUM already in [n, B] layout, m-tile by
    m-tile, letting AG1 start halfway through the layer.
"""

import contextlib
import ctypes
import os

import numpy as np
import ml_dtypes

NCORES = 8
PART = 128
B = 32
R = 1024
N = 12288
MSH = N // NCORES          # 1536 columns per core
MT = MSH // PART           # 12
NMC = MSH // 512           # 3 psum chunks
KC_R = R // PART           # 8
KC_N = N // PART           # 96
G = 2                      # AllGather chunks per boundary
BLK = MT // G              # 6 k-chunks per consumption block / weight slab
NSLAB = KC_N // BLK        # 16 pinned weight slabs
WS = 8.0                   # fp8 weight scale

bf16_np = ml_dtypes.bfloat16
fp8_np = ml_dtypes.float8_e4m3

_compiled_cache = {}


# --------------------------------------------------------------------------
# optional NTFF profiling hook (active only when BASS_KERNEL_PROFILE_DIR set)
# --------------------------------------------------------------------------
def _profile_ctx():
    out_dir = os.environ.get("BASS_KERNEL_PROFILE_DIR")
    if not out_dir:
        return contextlib.nullcontext()
    try:
        lib = ctypes.CDLL("/opt/axon/libaxon_pjrt.so")
        if not hasattr(lib, "axon_start_nrt_profile"):
            return contextlib.nullcontext()
        lib.axon_start_nrt_profile.argtypes = [
            ctypes.POINTER(ctypes.c_int64),
            ctypes.c_size_t,
        ]
        lib.axon_start_nrt_profile.restype = ctypes.c_int64
        lib.axon_stop_nrt_profile.argtypes = [ctypes.c_char_p]
        lib.axon_stop_nrt_profile.restype = ctypes.c_int64
    except OSError:
        return contextlib.nullcontext()

    @contextlib.contextmanager
    def _hook():
        import jax

        jax.devices()
        ids_env = os.environ.get("BASS_KERNEL_PROFILE_CORES", "")
        if ids_env:
            ids_list = [int(t) for t in ids_env.split(",") if t != ""]
            ids = (ctypes.c_int64 * len(ids_list))(*ids_list)
            rc = lib.axon_start_nrt_profile(ids, len(ids_list))
        else:
            rc = lib.axon_start_nrt_profile(None, 0)
        if rc != 0:
            raise RuntimeError(f"axon_start_nrt_profile rc={rc}")
        try:
            yield
        finally:
            os.makedirs(out_dir, exist_ok=True)
            n = lib.axon_stop_nrt_profile(str(out_dir).encode())
            print(f"profile: {n} file(s) written to {out_dir}")

    return _hook()


def _ensure_axon_platform():
    import jax

    try:
        devs = jax.devices()
    except Exception:
        devs = []
    if len(devs) >= NCORES and all("cpu" not in str(d).lower() for d in devs[:NCORES]):
        return
    import jax.extend.backend as jeb

    jeb.clear_backends()
    jax.config.update("jax_platforms", None)
    devs = jax.devices()
    if len(devs) < NCORES:
        raise RuntimeError(f"need {NCORES} neuron cores, got {devs}")


# --------------------------------------------------------------------------
# device program
# --------------------------------------------------------------------------
def _build_program(n_mid, scale_consts):
    """SPMD Bass program.  n_mid = number of full-N fp8 layers (>=1).
    scale_consts[i] = multiplier applied to layer i's psum on the way out
    (i=0 is the folded first layer; the last mid layer unscales to true)."""
    import concourse.bacc as bacc
    import concourse.tile as tile
    import concourse.mybir as mybir

    bf16 = mybir.dt.bfloat16
    fp8 = mybir.dt.float8e4
    f32 = mybir.dt.float32
    DR = mybir.MatmulPerfMode.DoubleRow

    nc = bacc.Bacc("TRN2", target_bir_lowering=False, debug=False,
                   num_devices=NCORES)

    xT_d = nc.dram_tensor("xT", [PART, KC_R * B], bf16, kind="ExternalInput")
    f_d = nc.dram_tensor("fw", [PART, KC_R * MSH], bf16, kind="ExternalInput")
    w2_d = nc.dram_tensor("w2", [NSLAB, PART, BLK * MSH], fp8,
                          kind="ExternalInput")
    m4_d = nc.dram_tensor("m4", [PART, MT * 2], bf16, kind="ExternalInput")
    y_d = nc.dram_tensor("y_part", [B, 2], f32, kind="ExternalOutput")

    # AllGather buffers: one per (boundary, chunk)
    ag_outs = [
        [nc.dram_tensor(f"ag{li}_{g}", [NCORES, PART, BLK * B], fp8,
                        addr_space="Shared") for g in range(G)]
        for li in range(n_mid)
    ]

    with tile.TileContext(nc) as tc:
        with (
            tc.tile_pool(name="const", bufs=1) as const,
            tc.tile_pool(name="wpin", bufs=1) as wpin,
            tc.tile_pool(name="acts", bufs=1) as acts,
            tc.tile_pool(name="psum", bufs=1, space="PSUM") as psum,
            tc.tile_pool(name="dram", bufs=1, space="DRAM") as dram,
        ):
            # ---- constant + weight DMAs (sync queue) ---------------------
            xT_sb = const.tile([PART, KC_R, B], bf16, name="xT_sb")
            nc.sync.dma_start(xT_sb.rearrange("p k b -> p (k b)"), xT_d[:])
            f_sb = const.tile([PART, KC_R, MSH], bf16, name="f_sb")
            nc.sync.dma_start(f_sb.rearrange("p k m -> p (k m)"), f_d[:])
            m4_sb = const.tile([PART, MT, 2], bf16, name="m4_sb")
            nc.sync.dma_start(m4_sb.rearrange("p t o -> p (t o)"), m4_d[:])
            wp = []
            for s in range(NSLAB):
                w = wpin.tile([PART, BLK, MSH], fp8, name=f"wp{s}",
                              tag=f"wp{s}")
                nc.sync.dma_start(w.rearrange("p k m -> p (k m)"), w2_d[s])
                wp.append(w)

            # ---- L1' (folded retina), transposed: psum[mt] = F_mt^T x ----
            # chunk tiles a1c[g] feed both the AG payload and nothing else
            a1c = [acts.tile([PART, BLK, B], fp8, name=f"a1c{g}")
                   for g in range(G)]
            ps1 = [psum.tile([PART, BLK, B], f32, name=f"ps1_{g}",
                             tag=f"ps1_{g}")
                   for g in range(G)]
            for g in range(G):
                for t in range(BLK):
                    mt = g * BLK + t
                    for kc in range(KC_R):
                        nc.tensor.matmul(
                            ps1[g][:, t, :],
                            f_sb[:, kc, mt * PART:(mt + 1) * PART],
                            xT_sb[:, kc, :],
                            start=(kc == 0), stop=(kc == KC_R - 1))
                nc.vector.tensor_scalar_mul(
                    a1c[g][:], ps1[g][:], float(scale_consts[0]))
                # stage + AllGather this chunk (scalar queue, CC on gpsimd)
                cc_in = dram.tile([PART, BLK * B], fp8, name=f"cc1_{g}",
                                  tag="cc1", bufs=G)
                nc.scalar.dma_start(
                    cc_in[:], a1c[g].rearrange("p t b -> p (t b)"))
                nc.gpsimd.collective_compute(
                    "AllGather", mybir.AluOpType.bypass,
                    replica_groups=[list(range(NCORES))],
                    ins=[cc_in.opt()], outs=[ag_outs[0][g][:]])

            # ---- mid layers ---------------------------------------------
            def load_rem(li):
                rem = []
                for g in range(G):
                    r_t = acts.tile([PART, NCORES, BLK, B], fp8,
                                    name=f"rem{li}_{g}", tag=f"rem_{g}",
                                    bufs=2)
                    nc.scalar.dma_start(
                        r_t.rearrange("p r t b -> p r t b"),
                        ag_outs[li][g].rearrange("r p (t b) -> p r t b", b=B))
                    rem.append(r_t)
                return rem

            psm = [psum.tile([B, 512], f32, name=f"psm{mc}", tag=f"psm{mc}")
                   for mc in range(NMC)]

            def mid_layer(li, rem, out_scale, last):
                n_pairs_tot = NSLAB * (BLK // 2)
                pi = 0
                for s in range(NSLAB):
                    g, r = divmod(s, NCORES)   # slab s <-> (chunk g, core r)
                    for j in range(BLK // 2):
                        lhsT = rem[g][:, r, 2 * j:2 * j + 2, :]
                        for mc in range(NMC):
                            nc.tensor.matmul(
                                psm[mc][:],
                                lhsT,
                                wp[s][:, 2 * j:2 * j + 2,
                                      mc * 512:(mc + 1) * 512],
                                start=(pi == 0),
                                stop=(pi == n_pairs_tot - 1),
                                perf_mode=DR)
                        pi += 1
                # epilogue: psum -> scaled bf16 -> transpose -> (fp8 + AG)
                y_sb = acts.tile([B, MSH], bf16, name="y_sb", tag="y_sb",
                                 bufs=2)
                aT_bf = acts.tile([PART, MT, B], bf16, name=f"aTb{li}",
                                  tag="aT_bf", bufs=2)
                for mc in range(NMC):
                    nc.vector.tensor_scalar_mul(
                        y_sb[:, mc * 512:(mc + 1) * 512], psm[mc][:],
                        float(out_scale))
                for g in range(G):
                    for t in range(BLK):
                        mt = g * BLK + t
                        for i in range(4):
                            nc.vector.transpose(
                                aT_bf[32 * i:32 * (i + 1), mt, :],
                                y_sb[:, mt * PART + 32 * i:
                                     mt * PART + 32 * (i + 1)])
                    if not last:
                        a_c = acts.tile([PART, BLK, B], fp8,
                                        name=f"a{li}c{g}", tag=f"a_c{g}",
                                        bufs=2)
                        nc.vector.tensor_copy(
                            a_c[:], aT_bf[:, g * BLK:(g + 1) * BLK, :])
                        cc_in = dram.tile([PART, BLK * B], fp8,
                                          name=f"cc{li}_{g}", tag="cc_mid",
                                          bufs=2 * G)
                        nc.scalar.dma_start(
                            cc_in[:], a_c.rearrange("p t b -> p (t b)"))
                        nc.gpsimd.collective_compute(
                            "AllGather", mybir.AluOpType.bypass,
                            replica_groups=[list(range(NCORES))],
                            ins=[cc_in.opt()], outs=[ag_outs[li + 1][g][:]])
                return aT_bf

            aT_last = None
            for li in range(n_mid):
                rem = load_rem(li)
                aT_last = mid_layer(li, rem, scale_consts[li + 1],
                                    last=(li == n_mid - 1))

            # ---- folded last layer + rational readout --------------------
            ps4 = psum.tile([B, 2], f32, name="ps4", tag="ps4")
            for t in range(MT):
                nc.tensor.matmul(
                    ps4[:], aT_last[:, t, :], m4_sb[:, t, :],
                    start=(t == 0), stop=(t == MT - 1))
            y_sb4 = acts.tile([B, 2], f32, name="y_sb4", tag="y4")
            nc.vector.tensor_copy(y_sb4[:], ps4[:])
            nc.sync.dma_start(y_d[:], y_sb4[:])

    nc.compile()
    return nc


# --------------------------------------------------------------------------
# host-side helpers
# --------------------------------------------------------------------------
def _to_dev_layout_2d(a, kc):
    """[kc*128, m] -> [128, kc*m] with out[p, k*m:(k+1)*m] = a[k*128+p, :]."""
    rows, m = a.shape
    assert rows == kc * PART
    return np.ascontiguousarray(
        a.reshape(kc, PART, m).transpose(1, 0, 2).reshape(PART, kc * m))


def _act_scales(F, W, n_mid):
    """Static power-of-2 activation scales from a weight-only probe."""
    rng = np.random.default_rng(12345)
    g = rng.standard_normal((2, F.shape[1])).astype(np.float32)
    rms = float(np.sqrt(np.mean(g ** 2)))
    a = g @ F.T
    amaxes = [float(np.abs(a).max()) / rms]
    for _ in range(n_mid - 1):
        a = a @ W.T
        amaxes.append(float(np.abs(a).max()) / rms)
    # margin 6x under e4m3 max 240
    return [2.0 ** np.floor(np.log2(240.0 / (6.0 * m))) for m in amaxes]


def kernel(x, W_retina, W_shared, W_rational, n_layers):
    x = np.asarray(x, np.float32)
    W_retina = np.asarray(W_retina, np.float32)
    W_shared = np.asarray(W_shared, np.float32)
    W_rational = np.asarray(W_rational, np.float32)
    L = int(n_layers)

    Bx, Rx = x.shape
    Nx = W_shared.shape[0]
    O = W_rational.shape[0]

    vis = np.flatnonzero(np.any(W_retina != 0, axis=1))
    rat = np.flatnonzero(np.any(W_rational != 0, axis=0))

    if (L < 3 or len(vis) == 0 or len(rat) == 0 or Nx != N or Rx != R
            or Bx != B or O != 2):
        out = x @ W_retina.T
        for _ in range(L):
            out = out @ W_shared.T
        return (out @ W_rational.T).astype(np.float32)

    n_mid = L - 2

    # ---- host folds ------------------------------------------------------
    F = W_shared[:, vis] @ np.ascontiguousarray(W_retina[vis, :])   # [N, R]
    M = (W_rational[:, rat].astype(np.float64)
         @ W_shared[rat, :].astype(np.float64)).astype(np.float32)  # [2, N]

    # ---- activation scale plan ------------------------------------------
    s = _act_scales(F, W_shared, n_mid)
    # psum multipliers: layer0 out *= s[0]; mid i out *= s[i+1]/(s[i]*WS);
    # last mid out *= 1/(s[n_mid-1]*WS)
    consts = [s[0]]
    for i in range(n_mid - 1):
        consts.append(s[i + 1] / (s[i] * WS))
    consts.append(1.0 / (s[n_mid - 1] * WS))

    # ---- per-core weight prep -------------------------------------------
    xT = _to_dev_layout_2d(np.ascontiguousarray(x.T).astype(bf16_np), KC_R)
    W8 = (W_shared.T * np.float32(WS)).astype(fp8_np)               # [N, N]

    # canonical consumption order: chunk g, core r -> rows
    # [r*MSH + g*BLK*PART : r*MSH + (g+1)*BLK*PART]
    row_order = np.concatenate([
        np.arange(r * MSH + g * BLK * PART, r * MSH + (g + 1) * BLK * PART)
        for g in range(G) for r in range(NCORES)
    ])

    f_c, w2_c, m4_c = [], [], []
    for c in range(NCORES):
        sl = slice(c * MSH, (c + 1) * MSH)
        f_c.append(_to_dev_layout_2d(
            np.ascontiguousarray(F[sl, :].T).astype(bf16_np), KC_R))
        Wc = W8[:, sl][row_order, :]                                # [N, MSH]
        w2_c.append(np.ascontiguousarray(
            Wc.reshape(NSLAB, BLK, PART, MSH).transpose(0, 2, 1, 3)
            .reshape(NSLAB, PART, BLK * MSH)))
        m4_c.append(_to_dev_layout_2d(
            np.ascontiguousarray(M[:, sl].T).astype(bf16_np), MT))

    _ensure_axon_platform()
    from concourse.bass_utils import run_bass_kernel_spmd

    key = (n_mid, tuple(consts))
    if key not in _compiled_cache:
        _compiled_cache[key] = _build_program(n_mid, consts)
    nc = _compiled_cache[key]

    in_maps = [
        {"xT": xT, "fw": f_c[c], "w2": w2_c[c], "m4": m4_c[c]}
        for c in range(NCORES)
    ]

    with _profile_ctx():
        res = run_bass_kernel_spmd(nc, in_maps, core_ids=list(range(NCORES)))

    y = np.zeros((B, O), np.float64)
    for c in range(NCORES):
        y += res.results[c]["y_part"].astype(np.float64)
    return y.astype(np.float32)


# revision 5
# speedup vs baseline: 1.6729x; 1.3041x over previous
"""Trainium2 Bass kernel for ConnectomeNetwork (gnn_message_passing).

Computation (reference):
    out = x @ W_retina^T                      # [B, N], nonzero only at visual cols
    for _ in range(n_layers): out = out @ W_shared^T
    y = out @ W_rational^T                    # [B, 2]

Strategy (8 NeuronCores, tensor-parallel over output columns):
  * Host folds:  F = W_shared[:, vis] @ W_retina[vis, :]   (retina + layer 1)
                 M = W_rational[:, rat] @ W_shared[rat, :] (layer L + rational)
    so the device runs L-1 dense layers: one bf16 layer contracting over
    R=1024 (F), then L-2 "mid" layers over the full N contraction, then a
    tiny folded readout.
  * Mid layers run in fp8 (e4m3) with DoubleRow matmuls: weights are scaled
    by 8, activations are rescaled per layer with static power-of-2 scales
    derived from a weight-only random probe.  Quantization noise injected
    before the last layer is strongly attenuated because W_shared's spectrum
    is dominated by its all-positive mean direction.
  * Each core owns a 1536-column shard of every layer; its fp8 weight shard
    (18.9 MB) streams into SBUF once and stays pinned for both mid layers.
    Activations are AllGathered (fp8 payload) between layers in 2 chunks,
    overlapped with compute where the dataflow allows.
  * The folded first layer is computed transposed (F stationary, xT moving)
    so activations come out of PSUM already in [n, B] layout, m-tile by
    m-tile, letting AG1 start halfway through the layer.
"""

import contextlib
import ctypes
import os

import numpy as np
import ml_dtypes

NCORES = 8
PART = 128
B = 32
R = 1024
N = 12288
MSH = N // NCORES          # 1536 columns per core
MT = MSH // PART           # 12
NMC = MSH // 512           # 3 psum chunks
KC_R = R // PART           # 8
KC_N = N // PART           # 96
G = 2                      # AllGather chunks per boundary
BLK = MT // G              # 6 k-chunks per consumption block / weight slab
NSLAB = KC_N // BLK        # 16 pinned weight slabs
WS = 8.0                   # fp8 weight scale

bf16_np = ml_dtypes.bfloat16
fp8_np = ml_dtypes.float8_e4m3

_compiled_cache = {}


# --------------------------------------------------------------------------
# optional NTFF profiling hook (active only when BASS_KERNEL_PROFILE_DIR set)
# --------------------------------------------------------------------------
def _profile_ctx():
    out_dir = os.environ.get("BASS_KERNEL_PROFILE_DIR")
    if not out_dir:
        return contextlib.nullcontext()
    try:
        lib = ctypes.CDLL("/opt/axon/libaxon_pjrt.so")
        if not hasattr(lib, "axon_start_nrt_profile"):
            return contextlib.nullcontext()
        lib.axon_start_nrt_profile.argtypes = [
            ctypes.POINTER(ctypes.c_int64),
            ctypes.c_size_t,
        ]
        lib.axon_start_nrt_profile.restype = ctypes.c_int64
        lib.axon_stop_nrt_profile.argtypes = [ctypes.c_char_p]
        lib.axon_stop_nrt_profile.restype = ctypes.c_int64
    except OSError:
        return contextlib.nullcontext()

    @contextlib.contextmanager
    def _hook():
        import jax

        jax.devices()
        ids_env = os.environ.get("BASS_KERNEL_PROFILE_CORES", "")
        if ids_env:
            ids_list = [int(t) for t in ids_env.split(",") if t != ""]
            ids = (ctypes.c_int64 * len(ids_list))(*ids_list)
            rc = lib.axon_start_nrt_profile(ids, len(ids_list))
        else:
            rc = lib.axon_start_nrt_profile(None, 0)
        if rc != 0:
            raise RuntimeError(f"axon_start_nrt_profile rc={rc}")
        try:
            yield
        finally:
            os.makedirs(out_dir, exist_ok=True)
            n = lib.axon_stop_nrt_profile(str(out_dir).encode())
            print(f"profile: {n} file(s) written to {out_dir}")

    return _hook()


def _ensure_axon_platform():
    import jax

    try:
        devs = jax.devices()
    except Exception:
        devs = []
    if len(devs) >= NCORES and all("cpu" not in str(d).lower() for d in devs[:NCORES]):
        return
    import jax.extend.backend as jeb

    jeb.clear_backends()
    jax.config.update("jax_platforms", None)
    devs = jax.devices()
    if len(devs) < NCORES:
        raise RuntimeError(f"need {NCORES} neuron cores, got {devs}")


# --------------------------------------------------------------------------
# device program
# --------------------------------------------------------------------------
def _build_program(n_mid, scale_consts):
    """SPMD Bass program.  n_mid = number of full-N fp8 layers (>=1).
    scale_consts[i] = multiplier applied to layer i's psum on the way out
    (i=0 is the folded first layer; the last mid layer unscales to true)."""
    import concourse.bacc as bacc
    import concourse.tile as tile
    import concourse.mybir as mybir

    bf16 = mybir.dt.bfloat16
    fp8 = mybir.dt.float8e4
    f32 = mybir.dt.float32
    DR = mybir.MatmulPerfMode.DoubleRow

    nc = bacc.Bacc("TRN2", target_bir_lowering=False, debug=False,
                   num_devices=NCORES)

    xT_d = nc.dram_tensor("xT", [PART, KC_R * B], bf16, kind="ExternalInput")
    f_d = nc.dram_tensor("fw", [PART, KC_R * MSH], bf16, kind="ExternalInput")
    w2_d = nc.dram_tensor("w2", [NSLAB, PART, BLK * MSH], fp8,
                          kind="ExternalInput")
    m4_d = nc.dram_tensor("m4", [PART, MT * 2], bf16, kind="ExternalInput")
    y_d = nc.dram_tensor("y_part", [B, 2], f32, kind="ExternalOutput")

    # AllGather buffers: one per (boundary, chunk)
    ag_outs = [
        [nc.dram_tensor(f"ag{li}_{g}", [NCORES, PART, BLK * B], fp8,
                        addr_space="Shared") for g in range(G)]
        for li in range(n_mid)
    ]

    with tile.TileContext(nc) as tc:
        with (
            tc.tile_pool(name="const", bufs=1) as const,
            tc.tile_pool(name="wpin", bufs=1) as wpin,
            tc.tile_pool(name="acts", bufs=1) as acts,
            tc.tile_pool(name="psum", bufs=1, space="PSUM") as psum,
            tc.tile_pool(name="dram", bufs=1, space="DRAM") as dram,
        ):
            # ---- constant + weight DMAs (sync queue) ---------------------
            xT_sb = const.tile([PART, KC_R, B], bf16, name="xT_sb")
            nc.sync.dma_start(xT_sb.rearrange("p k b -> p (k b)"), xT_d[:])
            f_sb = const.tile([PART, KC_R, MSH], bf16, name="f_sb")
            nc.sync.dma_start(f_sb.rearrange("p k m -> p (k m)"), f_d[:])
            m4_sb = const.tile([PART, MT, 2], bf16, name="m4_sb")
            nc.sync.dma_start(m4_sb.rearrange("p t o -> p (t o)"), m4_d[:])
            # prefetch only the first few weight slabs now; the rest are
            # emitted on the scalar queue after the AG1 loads so they cannot
            # contend with the first collective for HBM/DMA bandwidth
            PREF = 4
            wp = [wpin.tile([PART, BLK, MSH], fp8, name=f"wp{s}",
                            tag=f"wp{s}") for s in range(NSLAB)]
            for s in range(PREF):
                nc.sync.dma_start(wp[s].rearrange("p k m -> p (k m)"),
                                  w2_d[s])

            # ---- L1' (folded retina), transposed: psum[mt] = F_mt^T x ----
            # chunk tiles a1c[g] feed both the AG payload and nothing else
            a1c = [acts.tile([PART, BLK, B], fp8, name=f"a1c{g}")
                   for g in range(G)]
            ps1 = [psum.tile([PART, BLK, B], f32, name=f"ps1_{g}",
                             tag=f"ps1_{g}")
                   for g in range(G)]
            for g in range(G):
                for t in range(BLK):
                    mt = g * BLK + t
                    for kc in range(KC_R):
                        nc.tensor.matmul(
                            ps1[g][:, t, :],
                            f_sb[:, kc, mt * PART:(mt + 1) * PART],
                            xT_sb[:, kc, :],
                            start=(kc == 0), stop=(kc == KC_R - 1))
                nc.vector.tensor_scalar_mul(
                    a1c[g][:], ps1[g][:], float(scale_consts[0]))
                # stage + AllGather this chunk (scalar queue, CC on gpsimd)
                cc_in = dram.tile([PART, BLK * B], fp8, name=f"cc1_{g}",
                                  tag="cc1", bufs=G)
                nc.scalar.dma_start(
                    cc_in[:], a1c[g].rearrange("p t b -> p (t b)"))
                nc.gpsimd.collective_compute(
                    "AllGather", mybir.AluOpType.bypass,
                    replica_groups=[list(range(NCORES))],
                    ins=[cc_in.opt()], outs=[ag_outs[0][g][:]])

            # ---- mid layers ---------------------------------------------
            def load_rem(li):
                rem = []
                for g in range(G):
                    r_t = acts.tile([PART, NCORES, BLK, B], fp8,
                                    name=f"rem{li}_{g}", tag=f"rem_{g}",
                                    bufs=2)
                    nc.scalar.dma_start(
                        r_t.rearrange("p r t b -> p r t b"),
                        ag_outs[li][g].rearrange("r p (t b) -> p r t b", b=B))
                    rem.append(r_t)
                return rem

            psm = [psum.tile([B, 512], f32, name=f"psm{mc}", tag=f"psm{mc}")
                   for mc in range(NMC)]

            def mid_layer(li, rem, out_scale, last):
                n_pairs_tot = NSLAB * (BLK // 2)
                pi = 0
                for s in range(NSLAB):
                    g, r = divmod(s, NCORES)   # slab s <-> (chunk g, core r)
                    for j in range(BLK // 2):
                        lhsT = rem[g][:, r, 2 * j:2 * j + 2, :]
                        for mc in range(NMC):
                            nc.tensor.matmul(
                                psm[mc][:],
                                lhsT,
                                wp[s][:, 2 * j:2 * j + 2,
                                      mc * 512:(mc + 1) * 512],
                                start=(pi == 0),
                                stop=(pi == n_pairs_tot - 1),
                                perf_mode=DR)
                        pi += 1
                # epilogue: psum -> scaled bf16 -> transpose -> (fp8 + AG)
                y_sb = acts.tile([B, MSH], bf16, name="y_sb", tag="y_sb",
                                 bufs=2)
                aT_bf = acts.tile([PART, MT, B], bf16, name=f"aTb{li}",
                                  tag="aT_bf", bufs=2)
                for mc in range(NMC):
                    nc.vector.tensor_scalar_mul(
                        y_sb[:, mc * 512:(mc + 1) * 512], psm[mc][:],
                        float(out_scale))
                for g in range(G):
                    for t in range(BLK):
                        mt = g * BLK + t
                        for i in range(4):
                            nc.vector.transpose(
                                aT_bf[32 * i:32 * (i + 1), mt, :],
                                y_sb[:, mt * PART + 32 * i:
                                     mt * PART + 32 * (i + 1)])
                    if not last:
                        a_c = acts.tile([PART, BLK, B], fp8,
                                        name=f"a{li}c{g}", tag=f"a_c{g}",
                                        bufs=2)
                        nc.vector.tensor_copy(
                            a_c[:], aT_bf[:, g * BLK:(g + 1) * BLK, :])
                        cc_in = dram.tile([PART, BLK * B], fp8,
                                          name=f"cc{li}_{g}", tag="cc_mid",
                                          bufs=2 * G)
                        nc.scalar.dma_start(
                            cc_in[:], a_c.rearrange("p t b -> p (t b)"))
                        nc.gpsimd.collective_compute(
                            "AllGather", mybir.AluOpType.bypass,
                            replica_groups=[list(range(NCORES))],
                            ins=[cc_in.opt()], outs=[ag_outs[li + 1][g][:]])
                return aT_bf

            aT_last = None
            for li in range(n_mid):
                rem = load_rem(li)
                if li == 0:
                    # stream the remaining slabs behind the AG1 loads
                    for s in range(PREF, NSLAB):
                        nc.scalar.dma_start(
                            wp[s].rearrange("p k m -> p (k m)"), w2_d[s])
                aT_last = mid_layer(li, rem, scale_consts[li + 1],
                                    last=(li == n_mid - 1))

            # ---- folded last layer + rational readout --------------------
            ps4 = psum.tile([B, 2], f32, name="ps4", tag="ps4")
            for t in range(MT):
                nc.tensor.matmul(
                    ps4[:], aT_last[:, t, :], m4_sb[:, t, :],
                    start=(t == 0), stop=(t == MT - 1))
            y_sb4 = acts.tile([B, 2], f32, name="y_sb4", tag="y4")
            nc.vector.tensor_copy(y_sb4[:], ps4[:])
            nc.sync.dma_start(y_d[:], y_sb4[:])

    nc.compile()
    return nc


# --------------------------------------------------------------------------
# host-side helpers
# --------------------------------------------------------------------------
def _to_dev_layout_2d(a, kc):
    """[kc*128, m] -> [128, kc*m] with out[p, k*m:(k+1)*m] = a[k*128+p, :]."""
    rows, m = a.shape
    assert rows == kc * PART
    return np.ascontiguousarray(
        a.reshape(kc, PART, m).transpose(1, 0, 2).reshape(PART, kc * m))


def _act_scales(F, W, n_mid):
    """Static power-of-2 activation scales from a weight-only probe."""
    rng = np.random.default_rng(12345)
    g = rng.standard_normal((2, F.shape[1])).astype(np.float32)
    rms = float(np.sqrt(np.mean(g ** 2)))
    a = g @ F.T
    amaxes = [float(np.abs(a).max()) / rms]
    for _ in range(n_mid - 1):
        a = a @ W.T
        amaxes.append(float(np.abs(a).max()) / rms)
    # margin 6x under e4m3 max 240
    return [2.0 ** np.floor(np.log2(240.0 / (6.0 * m))) for m in amaxes]


def kernel(x, W_retina, W_shared, W_rational, n_layers):
    x = np.asarray(x, np.float32)
    W_retina = np.asarray(W_retina, np.float32)
    W_shared = np.asarray(W_shared, np.float32)
    W_rational = np.asarray(W_rational, np.float32)
    L = int(n_layers)

    Bx, Rx = x.shape
    Nx = W_shared.shape[0]
    O = W_rational.shape[0]

    vis = np.flatnonzero(np.any(W_retina != 0, axis=1))
    rat = np.flatnonzero(np.any(W_rational != 0, axis=0))

    if (L < 3 or len(vis) == 0 or len(rat) == 0 or Nx != N or Rx != R
            or Bx != B or O != 2):
        out = x @ W_retina.T
        for _ in range(L):
            out = out @ W_shared.T
        return (out @ W_rational.T).astype(np.float32)

    n_mid = L - 2

    # ---- host folds ------------------------------------------------------
    F = W_shared[:, vis] @ np.ascontiguousarray(W_retina[vis, :])   # [N, R]
    M = (W_rational[:, rat].astype(np.float64)
         @ W_shared[rat, :].astype(np.float64)).astype(np.float32)  # [2, N]

    # ---- activation scale plan ------------------------------------------
    s = _act_scales(F, W_shared, n_mid)
    # psum multipliers: layer0 out *= s[0]; mid i out *= s[i+1]/(s[i]*WS);
    # last mid out *= 1/(s[n_mid-1]*WS)
    consts = [s[0]]
    for i in range(n_mid - 1):
        consts.append(s[i + 1] / (s[i] * WS))
    consts.append(1.0 / (s[n_mid - 1] * WS))

    # ---- per-core weight prep -------------------------------------------
    xT = _to_dev_layout_2d(np.ascontiguousarray(x.T).astype(bf16_np), KC_R)
    W8 = (W_shared.T * np.float32(WS)).astype(fp8_np)               # [N, N]

    # canonical consumption order: chunk g, core r -> rows
    # [r*MSH + g*BLK*PART : r*MSH + (g+1)*BLK*PART]
    row_order = np.concatenate([
        np.arange(r * MSH + g * BLK * PART, r * MSH + (g + 1) * BLK * PART)
        for g in range(G) for r in range(NCORES)
    ])

    f_c, w2_c, m4_c = [], [], []
    for c in range(NCORES):
        sl = slice(c * MSH, (c + 1) * MSH)
        f_c.append(_to_dev_layout_2d(
            np.ascontiguousarray(F[sl, :].T).astype(bf16_np), KC_R))
        Wc = W8[:, sl][row_order, :]                                # [N, MSH]
        w2_c.append(np.ascontiguousarray(
            Wc.reshape(NSLAB, BLK, PART, MSH).transpose(0, 2, 1, 3)
            .reshape(NSLAB, PART, BLK * MSH)))
        m4_c.append(_to_dev_layout_2d(
            np.ascontiguousarray(M[:, sl].T).astype(bf16_np), MT))

    _ensure_axon_platform()
    from concourse.bass_utils import run_bass_kernel_spmd

    key = (n_mid, tuple(consts))
    if key not in _compiled_cache:
        _compiled_cache[key] = _build_program(n_mid, consts)
    nc = _compiled_cache[key]

    in_maps = [
        {"xT": xT, "fw": f_c[c], "w2": w2_c[c], "m4": m4_c[c]}
        for c in range(NCORES)
    ]

    with _profile_ctx():
        res = run_bass_kernel_spmd(nc, in_maps, core_ids=list(range(NCORES)))

    y = np.zeros((B, O), np.float64)
    for c in range(NCORES):
        y += res.results[c]["y_part"].astype(np.float64)
    return y.astype(np.float32)
